# revision 3
# baseline (speedup 1.0000x reference)
"""Trilinear (NiftyReg LINEAR/ZERO-boundary) image resampling on 8 Trainium2 cores.

Problem: inputs [2,144,144,144,2] f32, deformation [2,144,144,144,3] f32 with
absolute voxel coords in [0, 143).  Output [2,144,144,144,2].

Strategy (v2)
-------------
Sharding: 8 cores x (batch, x-quarter): core i handles batch i//4, output
x-slabs [36*(i%4), 36*(i%4+1)).  Each core receives its batch's full input
volume as a corner table (gathers are data-dependent across the whole volume)
plus its own deformation slice, and produces its own output slice.

The corner table is fp16: row v = the 16 corner values (4 xy-corners x 2 z x
2 ch) of voxel v, 32B.  One GPSIMD indirect DMA gathers 128 voxels' rows; this
instruction (SWDGE desc-gen, ~1.0-1.1us per 128 descriptors, Pool-engine
serial) is the hard bottleneck: 5832 instructions/core.  Everything else (DVE
index+weight math, fp16 interpolation, IO DMAs) is hidden behind it:
  - G (gathered rows), W8h (fp16 weights), IDX, OUT are triple-buffered so
    GPSIMD only ever waits for IDX availability (idxw runs ~2 chunks ahead).
  - interp consumes G/W8h one chunk behind the gathers.
Output is fp16, upcast on host.
"""

import sys

if "/opt/trn_rl_repo" not in sys.path:
    sys.path.insert(0, "/opt/trn_rl_repo")

import numpy as np

import concourse.bass as bass
import concourse.mybir as mybir
from concourse.bass import IndirectOffsetOnAxis

P = 128
S = 144
C = 2
B = 2
V = S * S * S              # voxels per batch volume
XS = 36                    # x-slabs per core
NVOX = XS * S * S          # voxels per core = 746496
NPP = NVOX // P            # voxels per partition = 5832
NCH = 12                   # chunks
N = NPP // NCH             # voxels per partition per chunk = 486
assert N * NCH == NPP
NB = 3                     # G/W8/IDX/OUT buffer depth

F32 = mybir.dt.float32
F16 = mybir.dt.float16
I32 = mybir.dt.int32
AL = mybir.AluOpType

# U-table: row v holds the 16 corner halfs of voxel v, layout [s=(dx,dy)][z][c]
U_OFFS = [0, S, S * S, S * S + S]  # voxel offsets of the 4 (dx,dy) corners


def build_u_table(vol: "np.ndarray") -> "np.ndarray":
    """vol [S,S,S,C] f32 -> U [V,16] f16: U[v] = corners (dx,dy,z,c) of voxel v."""
    vz = np.ascontiguousarray(vol.reshape(V, C))
    pad = S * S + S + 1
    Tt = np.zeros((V + pad, 4), np.float32)
    Tt[:V, 0:2] = vz
    Tt[: V - 1, 2:4] = vz[1:]
    U = np.empty((V, 16), np.float32)
    for s, off in enumerate(U_OFFS):
        U[:, 4 * s : 4 * s + 4] = Tt[off : V + off]
    return U.astype(np.float16)


def build_kernel() -> bass.Bass:
    nc = bass.Bass()

    volu = nc.dram_tensor("volu", [V, 16], F16, kind="ExternalInput")
    defm = nc.dram_tensor("defm", [NCH, P, N * 3], F32, kind="ExternalInput")
    outd = nc.dram_tensor("out", [NCH, P, N * 2], F16, kind="ExternalOutput")

    from contextlib import ExitStack

    with ExitStack() as ctx:
        DEF = ctx.enter_context(nc.sbuf_tensor("DEF", [P, 2, N, 3], F32))   # deformation chunk
        FR = ctx.enter_context(nc.sbuf_tensor("FR", [P, 3, N], F32))    # frac x,y,z planes
        OM = ctx.enter_context(nc.sbuf_tensor("OM", [P, 3, N], F32))    # 1-frac planes
        BS = ctx.enter_context(nc.sbuf_tensor("BS", [P, 3, N], F32))    # base x,y,z planes
        W8 = ctx.enter_context(nc.sbuf_tensor("W8", [P, 8, N], F32))    # f32 corner weights
        W8H = ctx.enter_context(nc.sbuf_tensor("W8H", [P, NB, 8, N], F16))  # fp16 weights
        T1 = ctx.enter_context(nc.sbuf_tensor("T1", [P, N], F32))       # scratch
        T2 = ctx.enter_context(nc.sbuf_tensor("T2", [P, N], F32))       # scratch
        IDX = ctx.enter_context(nc.sbuf_tensor("IDX", [P, NB, N], I32))     # flat voxel index
        I1 = ctx.enter_context(nc.sbuf_tensor("I1", [P, 3, N], I32))    # int base scratch
        G = ctx.enter_context(nc.sbuf_tensor("G", [P, NB, N, 16], F16))  # gathered corner rows
        M1 = ctx.enter_context(nc.sbuf_tensor("M1", [P, N, 2], F16))       # interp scratch
        M2 = ctx.enter_context(nc.sbuf_tensor("M2", [P, N, 2], F16))       # interp scratch
        M3 = ctx.enter_context(nc.sbuf_tensor("M3", [P, N, 2], F16))       # interp scratch
        OUT = ctx.enter_context(nc.sbuf_tensor("OUT", [P, NB, N, 2], F16))   # output chunk
        sem_def = (ctx.enter_context(nc.semaphore("sem_def0")), ctx.enter_context(nc.semaphore("sem_def1")))
        sem_g = ctx.enter_context(nc.semaphore("sem_g"))
        sem_out = ctx.enter_context(nc.semaphore("sem_out"))
        sem_idxw = ctx.enter_context(nc.semaphore("sem_idxw"))
        sem_int = ctx.enter_context(nc.semaphore("sem_int"))
        sem_dve = ctx.enter_context(nc.semaphore("sem_dve"))
        block = ctx.enter_context(nc.Block())

        def bcast(ap: bass.AP, extra: int) -> bass.AP:
            """Append a step-0 dim of size `extra` to an AP."""
            return bass.AP(ap.tensor, ap.offset, list(ap.ap) + [[0, extra]])

        @block.sync
        def _(sp):
            def store(cc: int):
                sp.wait_ge(sem_int, cc + 1)
                sp.dma_start(outd[cc], OUT[:, cc % NB]).then_inc(sem_out, 16)

            for c in range(NCH):
                k = c % 2
                if c >= 2:
                    sp.wait_ge(sem_idxw, c - 1)
                sp.dma_start(DEF[:, k], defm[c]).then_inc(sem_def[k], 16)
                if c >= 2:
                    store(c - 2)
            store(NCH - 2)
            store(NCH - 1)
            sp.wait_ge(sem_out, 16 * NCH)

        @block.vector
        def _(v):
            # Same-engine RAW on DVE needs explicit sync (deep pipeline): every
            # DVE op incs sem_dve; before each dependency level, wait for all
            # previously emitted DVE ops.
            dve_n = [0]

            def op(inst):
                inst.then_inc(sem_dve, 1)
                dve_n[0] += 1
                return inst

            def level():
                if dve_n[0] > 0:
                    v.wait_ge(sem_dve, dve_n[0])

            def idxw_phase(c: int):
                k = c % 2      # DEF buffer
                kb = c % NB    # IDX/W8H buffer
                j = c // 2
                v.wait_ge(sem_def[k], 16 * (j + 1))
                if c >= NB:
                    # IDX[kb]/W8H[kb] were last used by gathers/interp of chunk
                    # c-NB; gathers of chunk c-NB done once sem_g >= N*(c-NB+1)
                    v.wait_ge(sem_g, 16 * N * (c - NB + 1))
                    v.wait_ge(sem_int, c - NB + 1)
                level()
                # floor(d) robust to any f32->i32 rounding mode:
                #   i = cvt(d); r = cvt_f32(i); m = (r > d); base = r - m
                for ax in range(3):
                    op(v.tensor_copy(I1[:, ax], DEF[:, k, :, ax]))
                level()
                for ax in range(3):
                    op(v.tensor_copy(BS[:, ax], I1[:, ax]))
                level()
                for ax in range(3):
                    op(v.tensor_tensor(FR[:, ax], BS[:, ax], DEF[:, k, :, ax], AL.is_gt))
                level()
                for ax in range(3):
                    op(v.tensor_tensor(BS[:, ax], BS[:, ax], FR[:, ax], AL.subtract))
                level()
                # clamp base to <= 142 for OOB safety (no-op for valid inputs)
                op(v.tensor_scalar(BS[:], BS[:], float(S - 2), None, AL.min))
                level()
                for ax in range(3):
                    op(v.tensor_tensor(FR[:, ax], DEF[:, k, :, ax], BS[:, ax], AL.subtract))
                # idx = (bx*S + by)*S + bz   (exact in f32)
                op(v.scalar_tensor_tensor(T1[:], BS[:, 0], float(S), BS[:, 1], AL.mult, AL.add))
                level()
                # om = 1 - frac  ==  frac * -1 + 1
                op(v.tensor_scalar(OM[:], FR[:], -1.0, 1.0, AL.mult, AL.add))
                op(v.scalar_tensor_tensor(T2[:], T1[:], float(S), BS[:, 2], AL.mult, AL.add))
                level()
                # (dx,dy) corner weights into W8 planes 0,2,4,6 temporarily
                op(v.tensor_tensor(W8[:, 0], OM[:, 0], OM[:, 1], AL.mult))
                op(v.tensor_tensor(W8[:, 2], OM[:, 0], FR[:, 1], AL.mult))
                op(v.tensor_tensor(W8[:, 4], FR[:, 0], OM[:, 1], AL.mult))
                op(v.tensor_tensor(W8[:, 6], FR[:, 0], FR[:, 1], AL.mult))
                op(v.tensor_copy(IDX[:, kb], T2[:]))
                level()
                # fold z weight: plane 2s = w_s*(1-fz), plane 2s+1 = w_s*fz
                for s8 in (0, 2, 4, 6):
                    op(v.tensor_tensor(W8[:, s8 + 1], W8[:, s8], FR[:, 2], AL.mult))
                level()
                for s8 in (0, 2, 4, 6):
                    op(v.tensor_tensor(W8[:, s8], W8[:, s8], OM[:, 2], AL.mult))
                level()
                # downconvert weights to fp16 for the interp phase
                op(v.tensor_copy(W8H[:, kb], W8[:]))
                level()
                v.sem_inc(sem_idxw, 1)

            def interp_phase(c: int):
                kb = c % NB
                v.wait_ge(sem_g, 16 * N * (c + 1))
                if c >= NB:
                    # OUT[kb] was last read by the store DMA of chunk c-NB
                    v.wait_ge(sem_out, 16 * (c - NB + 1))
                level()

                def gs(e):  # G [P, N, 2] slice for corner pair e (0..7)
                    return G[:, kb, :, 2 * e : 2 * e + 2]

                def w(e):
                    return bcast(W8H[:, kb, e], 2)

                op(v.tensor_tensor(OUT[:, kb], gs(0), w(0), AL.mult))
                op(v.tensor_tensor(M1[:], gs(1), w(1), AL.mult))
                op(v.tensor_tensor(M2[:], gs(2), w(2), AL.mult))
                op(v.tensor_tensor(M3[:], gs(3), w(3), AL.mult))
                level()
                op(v.tensor_tensor(OUT[:, kb], OUT[:, kb], M1[:], AL.add))
                op(v.tensor_tensor(M2[:], M2[:], M3[:], AL.add))
                level()
                op(v.tensor_tensor(M1[:], gs(4), w(4), AL.mult))
                op(v.tensor_tensor(OUT[:, kb], OUT[:, kb], M2[:], AL.add))
                level()
                op(v.tensor_tensor(M2[:], gs(5), w(5), AL.mult))
                op(v.tensor_tensor(OUT[:, kb], OUT[:, kb], M1[:], AL.add))
                level()
                op(v.tensor_tensor(M1[:], gs(6), w(6), AL.mult))
                op(v.tensor_tensor(OUT[:, kb], OUT[:, kb], M2[:], AL.add))
                level()
                op(v.tensor_tensor(M2[:], gs(7), w(7), AL.mult))
                op(v.tensor_tensor(OUT[:, kb], OUT[:, kb], M1[:], AL.add))
                level()
                op(v.tensor_tensor(OUT[:, kb], OUT[:, kb], M2[:], AL.add))
                level()
                v.sem_inc(sem_int, 1)

            for c in range(NCH):
                idxw_phase(c)
                if c >= 1:
                    interp_phase(c - 1)
            interp_phase(NCH - 1)

        @block.gpsimd
        def _(g):
            volu_flat = volu[:]  # [V, 16] f16
            for c in range(NCH):
                kb = c % NB
                g.wait_ge(sem_idxw, c + 1)
                for jj in range(N):
                    nc.gpsimd.indirect_dma_start(
                        out=G[:, kb, jj],
                        out_offset=None,
                        in_=volu_flat,
                        in_offset=IndirectOffsetOnAxis(ap=IDX[:, kb, jj : jj + 1], axis=0),
                        element_offset=0,
                    ).then_inc(sem_g, 16)

    return nc


_NC_CACHE = None


def _get_nc():
    global _NC_CACHE
    if _NC_CACHE is None:
        _NC_CACHE = build_kernel()
    return _NC_CACHE


def _in_maps(inputs: np.ndarray, deformation: np.ndarray):
    u_tables = [build_u_table(inputs[b]) for b in range(B)]
    maps = []
    for core in range(8):
        b, q = core // 4, core % 4
        defc = np.ascontiguousarray(
            deformation[b, XS * q : XS * (q + 1)].reshape(NCH, P, N * 3)
        )
        maps.append({"volu": u_tables[b], "defm": defc})
    return maps


def run(inputs: np.ndarray, deformation: np.ndarray, trace: bool = False):
    from concourse.bass_utils import run_bass_kernel_spmd

    nc = _get_nc()
    res = run_bass_kernel_spmd(
        nc,
        _in_maps(inputs, deformation),
        core_ids=list(range(8)),
        trace=trace,
    )
    out = np.empty((B, S, S, S, C), dtype=np.float32)
    for core in range(8):
        b, q = core // 4, core % 4
        out[b, XS * q : XS * (q + 1)] = (
            res.results[core]["out"].astype(np.float32).reshape(XS, S, S, C)
        )
    return out, res


def kernel(inputs: np.ndarray, deformation: np.ndarray) -> np.ndarray:
    out, _ = run(np.asarray(inputs), np.asarray(deformation))
    return out


# revision 4
# speedup vs baseline: 1.1788x; 1.1788x over previous
"""Trilinear (NiftyReg LINEAR/ZERO-boundary) image resampling on 8 Trainium2 cores.

Problem: inputs [2,144,144,144,2] f32, deformation [2,144,144,144,3] f32 with
absolute voxel coords in [0, 143).  Output [2,144,144,144,2].

Strategy (v3)
-------------
Sharding: 8 cores x (batch, x-quarter): core i handles batch i//4, output
x-slabs [36*(i%4), 36*(i%4+1)).  Each core receives its batch's full input
volume as a corner table (gathers are data-dependent across the whole volume)
plus its own deformation slice, and produces its own output slice.

The corner table is fp16: row v = the 16 corner values (4 xy-corners x 2 z x
2 ch) of voxel v, 32B.  One GPSIMD indirect DMA gathers 128 voxels' rows; this
instruction (SWDGE desc-gen, ~1.1us + ~0.3us gap per 128 descriptors,
Pool-engine serial) is the hard bottleneck: 5832 instructions/core.  v3 keeps
the SDMA engines free of everything else while the gather stream runs:
  - the whole deformation slice (70KB/partition) is DMA'd into SBUF up front,
  - the whole output (23KB/partition, fp16) is buffered in SBUF and stored
    once at the end,
so no direct2d DMA competes with gather-descriptor drain mid-stream.  DVE
index+weight math runs one chunk ahead of the gathers; fp16 interpolation one
chunk behind.  Output is upcast on host.
"""

import sys

if "/opt/trn_rl_repo" not in sys.path:
    sys.path.insert(0, "/opt/trn_rl_repo")

import numpy as np

import concourse.bass as bass
import concourse.mybir as mybir
from concourse.bass import IndirectOffsetOnAxis

P = 128
S = 144
C = 2
B = 2
V = S * S * S              # voxels per batch volume
XS = 36                    # x-slabs per core
NVOX = XS * S * S          # voxels per core = 746496
NPP = NVOX // P            # voxels per partition = 5832
NCH = 12                   # chunks
N = NPP // NCH             # voxels per partition per chunk = 486
assert N * NCH == NPP

F32 = mybir.dt.float32
F16 = mybir.dt.float16
I32 = mybir.dt.int32
AL = mybir.AluOpType

# U-table: row v holds the 16 corner halfs of voxel v, layout [s=(dx,dy)][z][c]
U_OFFS = [0, S, S * S, S * S + S]  # voxel offsets of the 4 (dx,dy) corners


def build_u_table(vol: "np.ndarray") -> "np.ndarray":
    """vol [S,S,S,C] f32 -> U [V,16] f16: U[v] = corners (dx,dy,z,c) of voxel v."""
    vz = np.ascontiguousarray(vol.reshape(V, C))
    pad = S * S + S + 1
    Tt = np.zeros((V + pad, 4), np.float32)
    Tt[:V, 0:2] = vz
    Tt[: V - 1, 2:4] = vz[1:]
    U = np.empty((V, 16), np.float32)
    for s, off in enumerate(U_OFFS):
        U[:, 4 * s : 4 * s + 4] = Tt[off : V + off]
    return U.astype(np.float16)


def build_kernel() -> bass.Bass:
    nc = bass.Bass()

    volu = nc.dram_tensor("volu", [V, 16], F16, kind="ExternalInput")
    defm = nc.dram_tensor("defm", [P, NCH, N * 3], F32, kind="ExternalInput")
    outd = nc.dram_tensor("out", [P, NCH, N * 2], F16, kind="ExternalOutput")

    from contextlib import ExitStack

    with ExitStack() as ctx:
        DEF = ctx.enter_context(nc.sbuf_tensor("DEF", [P, NCH, N, 3], F32))  # whole deformation slice
        OUTA = ctx.enter_context(nc.sbuf_tensor("OUTA", [P, NCH, N, 2], F16))  # whole output
        FR = ctx.enter_context(nc.sbuf_tensor("FR", [P, 3, N], F32))    # frac x,y,z planes
        OM = ctx.enter_context(nc.sbuf_tensor("OM", [P, 3, N], F32))    # 1-frac planes
        BS = ctx.enter_context(nc.sbuf_tensor("BS", [P, 3, N], F32))    # base x,y,z planes
        W8 = ctx.enter_context(nc.sbuf_tensor("W8", [P, 8, N], F32))    # f32 corner weights
        W8H = ctx.enter_context(nc.sbuf_tensor("W8H", [P, 2, 8, N], F16))  # fp16 weights
        T1 = ctx.enter_context(nc.sbuf_tensor("T1", [P, N], F32))       # scratch
        T2 = ctx.enter_context(nc.sbuf_tensor("T2", [P, N], F32))       # scratch
        IDX = ctx.enter_context(nc.sbuf_tensor("IDX", [P, 2, N], I32))      # flat voxel index
        I1 = ctx.enter_context(nc.sbuf_tensor("I1", [P, 3, N], I32))    # int base scratch
        G = ctx.enter_context(nc.sbuf_tensor("G", [P, 2, N, 16], F16))  # gathered corner rows
        M1 = ctx.enter_context(nc.sbuf_tensor("M1", [P, N, 2], F16))       # interp scratch
        M2 = ctx.enter_context(nc.sbuf_tensor("M2", [P, N, 2], F16))       # interp scratch
        M3 = ctx.enter_context(nc.sbuf_tensor("M3", [P, N, 2], F16))       # interp scratch
        sem_def = ctx.enter_context(nc.semaphore("sem_def"))
        sem_g = ctx.enter_context(nc.semaphore("sem_g"))
        sem_out = ctx.enter_context(nc.semaphore("sem_out"))
        sem_idxw = ctx.enter_context(nc.semaphore("sem_idxw"))
        sem_int = ctx.enter_context(nc.semaphore("sem_int"))
        sem_dve = ctx.enter_context(nc.semaphore("sem_dve"))
        block = ctx.enter_context(nc.Block())

        def bcast(ap: bass.AP, extra: int) -> bass.AP:
            """Append a step-0 dim of size `extra` to an AP."""
            return bass.AP(ap.tensor, ap.offset, list(ap.ap) + [[0, extra]])

        @block.sync
        def _(sp):
            # chunk 0 first so idxw(0) starts early, then the rest
            sp.dma_start(DEF[:, 0], defm[:, 0]).then_inc(sem_def, 16)
            sp.dma_start(DEF[:, 1:], defm[:, 1:]).then_inc(sem_def, 16)
            sp.wait_ge(sem_int, NCH)
            sp.dma_start(outd[:], OUTA[:]).then_inc(sem_out, 16)
            sp.wait_ge(sem_out, 16)

        @block.vector
        def _(v):
            # Same-engine RAW on DVE needs explicit sync (deep pipeline): every
            # DVE op incs sem_dve; before each dependency level, wait for all
            # previously emitted DVE ops.
            dve_n = [0]

            def op(inst):
                inst.then_inc(sem_dve, 1)
                dve_n[0] += 1
                return inst

            def level():
                if dve_n[0] > 0:
                    v.wait_ge(sem_dve, dve_n[0])

            def idxw_phase(c: int):
                kb = c % 2    # IDX/W8H buffer
                v.wait_ge(sem_def, 16 if c == 0 else 32)
                # IDX[kb] reuse (gathers of c-2) and W8H[kb] reuse (interp of
                # c-2) are guarded transitively: interp(c-2) precedes this
                # phase in DVE program order and waits those gathers itself.
                level()
                # floor(d) robust to any f32->i32 rounding mode:
                #   i = cvt(d); r = cvt_f32(i); m = (r > d); base = r - m
                for ax in range(3):
                    op(v.tensor_copy(I1[:, ax], DEF[:, c, :, ax]))
                level()
                for ax in range(3):
                    op(v.tensor_copy(BS[:, ax], I1[:, ax]))
                level()
                for ax in range(3):
                    op(v.tensor_tensor(FR[:, ax], BS[:, ax], DEF[:, c, :, ax], AL.is_gt))
                level()
                for ax in range(3):
                    op(v.tensor_tensor(BS[:, ax], BS[:, ax], FR[:, ax], AL.subtract))
                level()
                # clamp base to <= 142 for OOB safety (no-op for valid inputs)
                op(v.tensor_scalar(BS[:], BS[:], float(S - 2), None, AL.min))
                level()
                for ax in range(3):
                    op(v.tensor_tensor(FR[:, ax], DEF[:, c, :, ax], BS[:, ax], AL.subtract))
                # idx = (bx*S + by)*S + bz   (exact in f32)
                op(v.scalar_tensor_tensor(T1[:], BS[:, 0], float(S), BS[:, 1], AL.mult, AL.add))
                level()
                # om = 1 - frac  ==  frac * -1 + 1
                op(v.tensor_scalar(OM[:], FR[:], -1.0, 1.0, AL.mult, AL.add))
                op(v.scalar_tensor_tensor(T2[:], T1[:], float(S), BS[:, 2], AL.mult, AL.add))
                level()
                # (dx,dy) corner weights into W8 planes 0,2,4,6 temporarily
                op(v.tensor_tensor(W8[:, 0], OM[:, 0], OM[:, 1], AL.mult))
                op(v.tensor_tensor(W8[:, 2], OM[:, 0], FR[:, 1], AL.mult))
                op(v.tensor_tensor(W8[:, 4], FR[:, 0], OM[:, 1], AL.mult))
                op(v.tensor_tensor(W8[:, 6], FR[:, 0], FR[:, 1], AL.mult))
                op(v.tensor_copy(IDX[:, kb], T2[:]))
                level()
                # fold z weight: plane 2s = w_s*(1-fz), plane 2s+1 = w_s*fz
                for s8 in (0, 2, 4, 6):
                    op(v.tensor_tensor(W8[:, s8 + 1], W8[:, s8], FR[:, 2], AL.mult))
                level()
                for s8 in (0, 2, 4, 6):
                    op(v.tensor_tensor(W8[:, s8], W8[:, s8], OM[:, 2], AL.mult))
                level()
                # downconvert weights to fp16 for the interp phase
                op(v.tensor_copy(W8H[:, kb], W8[:]))
                level()
                v.sem_inc(sem_idxw, 1)

            def interp_phase(c: int):
                kb = c % 2
                v.wait_ge(sem_g, 16 * N * (c + 1))
                level()

                def gs(e):  # G [P, N, 2] slice for corner pair e (0..7)
                    return G[:, kb, :, 2 * e : 2 * e + 2]

                def w(e):
                    return bcast(W8H[:, kb, e], 2)

                O = OUTA[:, c]
                op(v.tensor_tensor(O, gs(0), w(0), AL.mult))
                op(v.tensor_tensor(M1[:], gs(1), w(1), AL.mult))
                op(v.tensor_tensor(M2[:], gs(2), w(2), AL.mult))
                op(v.tensor_tensor(M3[:], gs(3), w(3), AL.mult))
                level()
                op(v.tensor_tensor(O, O, M1[:], AL.add))
                op(v.tensor_tensor(M2[:], M2[:], M3[:], AL.add))
                level()
                op(v.tensor_tensor(M1[:], gs(4), w(4), AL.mult))
                op(v.tensor_tensor(O, O, M2[:], AL.add))
                level()
                op(v.tensor_tensor(M2[:], gs(5), w(5), AL.mult))
                op(v.tensor_tensor(O, O, M1[:], AL.add))
                level()
                op(v.tensor_tensor(M1[:], gs(6), w(6), AL.mult))
                op(v.tensor_tensor(O, O, M2[:], AL.add))
                level()
                op(v.tensor_tensor(M2[:], gs(7), w(7), AL.mult))
                op(v.tensor_tensor(O, O, M1[:], AL.add))
                level()
                op(v.tensor_tensor(O, O, M2[:], AL.add))
                level()
                v.sem_inc(sem_int, 1)

            for c in range(NCH):
                idxw_phase(c)
                if c >= 1:
                    interp_phase(c - 1)
            interp_phase(NCH - 1)

        @block.gpsimd
        def _(g):
            volu_flat = volu[:]  # [V, 16] f16
            for c in range(NCH):
                kb = c % 2
                g.wait_ge(sem_idxw, c + 1)
                if c >= 2:
                    # G[kb] was last read by interp(c-2)
                    g.wait_ge(sem_int, c - 1)
                for jj in range(N):
                    nc.gpsimd.indirect_dma_start(
                        out=G[:, kb, jj],
                        out_offset=None,
                        in_=volu_flat,
                        in_offset=IndirectOffsetOnAxis(ap=IDX[:, kb, jj : jj + 1], axis=0),
                        element_offset=0,
                    ).then_inc(sem_g, 16)

    return nc


_NC_CACHE = None


def _get_nc():
    global _NC_CACHE
    if _NC_CACHE is None:
        _NC_CACHE = build_kernel()
    return _NC_CACHE


def _in_maps(inputs: np.ndarray, deformation: np.ndarray):
    u_tables = [build_u_table(inputs[b]) for b in range(B)]
    maps = []
    for core in range(8):
        b, q = core // 4, core % 4
        # [NCH, P, N, 3] voxel order -> [P, NCH, N*3] for the one-shot load
        d4 = deformation[b, XS * q : XS * (q + 1)].reshape(NCH, P, N, 3)
        defc = np.ascontiguousarray(d4.transpose(1, 0, 2, 3)).reshape(P, NCH, N * 3)
        maps.append({"volu": u_tables[b], "defm": defc})
    return maps


def run(inputs: np.ndarray, deformation: np.ndarray, trace: bool = False):
    from concourse.bass_utils import run_bass_kernel_spmd

    nc = _get_nc()
    res = run_bass_kernel_spmd(
        nc,
        _in_maps(inputs, deformation),
        core_ids=list(range(8)),
        trace=trace,
    )
    out = np.empty((B, S, S, S, C), dtype=np.float32)
    for core in range(8):
        b, q = core // 4, core % 4
        o = res.results[core]["out"].reshape(P, NCH, N, 2).transpose(1, 0, 2, 3)
        out[b, XS * q : XS * (q + 1)] = (
            o.astype(np.float32).reshape(XS, S, S, C)
        )
    return out, res


def kernel(inputs: np.ndarray, deformation: np.ndarray) -> np.ndarray:
    out, _ = run(np.asarray(inputs), np.asarray(deformation))
    return out
